# revision 1
# baseline (speedup 1.0000x reference)
"""DINN forward kernel for Trainium2 (Bass/Tile), batch-sharded across 8 NeuronCores.

Reference computation (B=16384, D=512):
    gates  = sigmoid(x @ W.T + b)                       # [B, D]
    linear = sum(gates * x, axis=1)                     # [B]
    quad   = sum_{i<j} iw_ij * x_i * x_j                # [B]
    out    = sigmoid(linear + quad)[:, None]            # [B, 1]

Data-parallel sharding: x is split along the batch across the 8 cores;
W, b and the (strictly upper-triangular) interaction matrix U built from iw
are replicated. No collectives are needed in the forward pass.

Per-core kernel (batch shard of 2048 rows), all matmuls in the "transposed"
orientation with the contraction dim D on SBUF partitions (host pre-transposes
x -> xT so no on-chip transposes are needed):
    G^T[dout, b] = sum_k Wt[k, dout] xT[k, b]   (f32r matmuls, 1 cy/row)
    T^T[dout, b] = sum_k U[k, dout] xT[k, b]    (fp32 exact; U strictly upper
                                                 -> the 6/16 lower blocks are
                                                 skipped entirely)
    sig = sigmoid(G^T + b)     on ACT (per-partition bias)
    P   = (sig + T^T) * xT     on DVE (two grouped 4-bank ops per batch tile)
    q   = ones^T @ P           fp32 PE matmul reduces P over partitions
The kernel returns the log-odds; the final sigmoid is applied on host in
float64 (exact, and the kernel's output stays well-conditioned).

Precision (hardware-measured): float32r matmuls carry ~13-bit mantissas (the
f32r DMA itself rounds), fine for the gates (log-odds error ~1e-3) but not for
the quad term, whose |log-odds| ~ 360 scale needs exact fp32 products. This
mix gives max abs output error ~4e-4 vs the fp32 reference.
"""
import sys

if "/opt/trn_rl_repo" not in sys.path:
    sys.path.insert(0, "/opt/trn_rl_repo")

import numpy as np

import concourse.tile as tile
from concourse import bacc, bass_isa, mybir
from concourse.bass_utils import run_bass_kernel_spmd

B, D = 16384, 512
NCORES = 8
BC = B // NCORES            # 2048 rows per core
NBT = BC // 512             # 4 batch tiles per core
NK = D // 128               # 4 contraction chunks

f32 = mybir.dt.float32
f32r = mybir.dt.float32r
AF = mybir.ActivationFunctionType

_CACHE = {}


def _build():
    nc = bacc.Bacc("TRN2", target_bir_lowering=False, debug=False,
                   num_devices=NCORES)

    d_xTf = nc.declare_dram_parameter("xTf", [D, BC], f32, isOutput=False)
    d_Wtr = nc.declare_dram_parameter("Wtr", [D, D], f32r, isOutput=False)
    d_Uf = nc.declare_dram_parameter("Uf", [D, D], f32, isOutput=False)
    d_bias = nc.declare_dram_parameter("bias", [D], f32, isOutput=False)
    d_out = nc.declare_dram_parameter("out", [1, BC], f32, isOutput=True)

    rearr = lambda ap: ap.rearrange("(c p) n -> p c n", p=128)

    with tile.TileContext(nc) as tc:
        with tc.tile_pool(name="const", bufs=1) as const, \
             tc.tile_pool(name="xin", bufs=2) as xin, \
             tc.tile_pool(name="elt", bufs=2) as elt, \
             tc.tile_pool(name="pg", bufs=3, space="PSUM") as pg, \
             tc.tile_pool(name="pt", bufs=1, space="PSUM") as pt:

            # ---- weights / constants (loaded once, replicated per core) ----
            Wtr_sb = const.tile([128, NK, D], f32r, tag="wtr")
            Uf_sb = const.tile([128, NK, D], f32, tag="uf")
            bias_sb = const.tile([128, NK], f32, tag="bias")
            out_sb = const.tile([1, BC], f32, tag="out_sb")

            xTf_r = rearr(d_xTf[:, :])
            # f32r view of the same fp32 dram tensor (no duplicate upload);
            # the f32r DMA rounds, which the gates tolerate.
            xTr_r = rearr(d_xTf[:, :].bitcast(f32r))
            Wtr_d = rearr(d_Wtr[:, :])

            # first transfers in need order, gates k=0 pieces first, so the
            # PE starts ~10us earlier instead of waiting for whole tensors
            xf0 = xin.tile([128, NK, 512], f32, tag="xf")
            xr0 = xin.tile([128, NK, 512], f32r, tag="xr")
            Uf_d = rearr(d_Uf[:, :])
            nc.sync.dma_start(out=xr0[:, 0, :], in_=xTr_r[:, 0, 0:512])
            nc.sync.dma_start(out=Wtr_sb[:, 0, :], in_=Wtr_d[:, 0, :])
            nc.sync.dma_start(out=xr0[:, 1:NK, :], in_=xTr_r[:, 1:NK, 0:512])
            nc.sync.dma_start(out=Wtr_sb[:, 1:NK, :], in_=Wtr_d[:, 1:NK, :])
            nc.sync.dma_start(out=xf0[:, 0, :], in_=xTf_r[:, 0, 0:512])
            nc.sync.dma_start(out=Uf_sb[:, 0, :], in_=Uf_d[:, 0, :])
            nc.sync.dma_start(
                out=bias_sb, in_=d_bias[:].rearrange("(c p) -> p c", p=128))
            nc.sync.dma_start(out=xf0[:, 1:NK, :], in_=xTf_r[:, 1:NK, 0:512])
            nc.sync.dma_start(out=Uf_sb[:, 1:NK, :], in_=Uf_d[:, 1:NK, :])

            for b0 in range(NBT):
                bsl = slice(b0 * 512, (b0 + 1) * 512)
                if b0 == 0:
                    xf, xr = xf0, xr0
                else:
                    xf = xin.tile([128, NK, 512], f32, tag="xf")
                    xr = xin.tile([128, NK, 512], f32r, tag="xr")
                    nc.sync.dma_start(out=xr, in_=xTr_r[:, :, bsl])
                    nc.sync.dma_start(out=xf, in_=xTf_r[:, :, bsl])

                pt_all = pt.tile([128, NK, 512], f32, tag="pt")   # 4 banks
                sig_all = elt.tile([128, NK, 512], f32, tag="sig")

                for m in range(NK):
                    msl = slice(m * 128, (m + 1) * 128)
                    # gates: G^T chunk, f32r (1 cy/row)
                    psum_g = pg.tile([128, 512], f32, tag="pg")
                    for k in range(NK):
                        nc.tensor.matmul(
                            psum_g, Wtr_sb[:, k, msl], xr[:, k, :],
                            start=(k == 0), stop=(k == NK - 1))

                    # quad: T^T chunk, exact fp32; skip zero blocks (k > m)
                    tdst = pt_all[:, m, :]
                    for k in range(m + 1):
                        nc.tensor.matmul(
                            tdst, Uf_sb[:, k, msl], xf[:, k, :],
                            start=(k == 0), stop=(k == m))

                    nc.scalar.activation(sig_all[:, m, :], psum_g, AF.Sigmoid,
                                         bias=bias_sb[:, m:m + 1], scale=1.0)

                # P = (sig + T^T) * xT.  For the last batch tile, split per
                # m-chunk so the DVE drain overlaps the final matmuls; for the
                # others the two grouped ops are cheaper (less per-op overhead)
                if b0 == NBT - 1:
                    p_all = elt.tile([128, NK, 512], f32, tag="p")
                    for m in range(NK):
                        s2m = elt.tile([128, 512], f32, tag="s2s")
                        nc.vector.tensor_add(s2m, sig_all[:, m, :],
                                             pt_all[:, m, :])
                        # multiply on GpSimd: pipelines against the DVE adds
                        # in the final tile's drain (operands are SBUF-only)
                        nc.gpsimd.tensor_mul(p_all[:, m, :], s2m, xf[:, m, :])
                else:
                    s2 = elt.tile([128, NK, 512], f32, tag="s2")
                    nc.vector.tensor_add(s2, sig_all, pt_all)
                    p_all = elt.tile([128, NK, 512], f32, tag="p")
                    nc.vector.tensor_mul(p_all, s2, xf)

                # reduce P over partitions on the (otherwise idle) GpSimd
                # engine: 2 DVE adds fold the 4 m-chunks, then one
                # partition_all_reduce; keeps all 16 ones-matmuls off the PE
                s4 = elt.tile([128, 2, 512], f32, tag="s4")
                nc.vector.tensor_add(s4, p_all[:, 0:2, :], p_all[:, 2:4, :])
                sred = elt.tile([128, 512], f32, tag="sred")
                nc.vector.tensor_add(sred, s4[:, 0, :], s4[:, 1, :])
                par = elt.tile([128, 512], f32, tag="par")
                nc.gpsimd.partition_all_reduce(par, sred, 128,
                                               bass_isa.ReduceOp.add)
                # evacuate log-odds (final sigmoid happens on host, exact);
                # last tile: DMA straight from par row 0, skipping the ACT
                if b0 == NBT - 1:
                    nc.sync.dma_start(out=d_out[0:1, bsl], in_=par[0:1, :])
                else:
                    nc.scalar.activation(out_sb[0:1, bsl], par[0:1, :],
                                         AF.Identity)

            nc.sync.dma_start(out=d_out[0:1, 0:(NBT - 1) * 512],
                              in_=out_sb[0:1, 0:(NBT - 1) * 512])

    nc.compile()
    return nc


def kernel(x, W, b, iw):
    x = np.asarray(x, np.float32)
    W = np.asarray(W, np.float32)
    b = np.asarray(b, np.float32)
    iw = np.asarray(iw, np.float32)

    # host prep: strictly upper-triangular U from iw (row-major i<j order),
    # pre-transposed operands so the contraction dim lands on SBUF partitions
    U = np.zeros((D, D), np.float32)
    iu, ju = np.triu_indices(D, k=1)
    U[iu, ju] = iw
    Wt = np.ascontiguousarray(W.T)          # lhsT for gates: Wt[k, dout]
    xT = x.T                                 # [D, B] view

    shared = {"Wtr": Wt, "Uf": U, "bias": b}
    in_maps = []
    for c in range(NCORES):
        m = dict(shared)
        m["xTf"] = np.ascontiguousarray(xT[:, c * BC:(c + 1) * BC])
        in_maps.append(m)

    if "nc" not in _CACHE:
        _CACHE["nc"] = _build()
    nc = _CACHE["nc"]

    res = run_bass_kernel_spmd(nc, in_maps, list(range(NCORES)))
    lo = np.concatenate(
        [res.results[c]["out"][0] for c in range(NCORES)]).astype(np.float64)
    out = 1.0 / (1.0 + np.exp(-np.clip(lo, -708.0, 708.0)))
    return out.reshape(B, 1).astype(np.float32)

